# revision 1
# baseline (speedup 1.0000x reference)
"""Bass/Trainium2 kernel for nn_BellmanLoss (8-core data-parallel).

Math: the reference's scatter makes Q_new differ from Q0 only at
a_i = argmax_j(actions[i, j]) (first max), so

    loss = sum_i (Q0[i, a_i] - target_i)^2
    target_i = r_i + 0.9 * max_a Qn[i, a] * (1 - done_i),   done_i = (states1[i,0] == 666)

Per core: 8192 rows. MLP runs feature-major (h^T = [features, batch]) so the
weights are the stationary matmul operand; Q0^T/Qn^T land stacked in one PSUM
tile ([0:64) and [64:96) partition groups via col-group tile_position), get
PE-transposed to batch-major, and a fused vector epilogue computes the
argmax-select, max_a, target and per-partition loss partials. Host does
layout-only prep (transpose/reshape/cast) and the final 1024-element sum.
"""

import os
import numpy as np

import concourse.bass as bass
import concourse.mybir as mybir
import concourse.tile as tile
from concourse import bacc
from concourse.bass_utils import run_bass_kernel_spmd

# Problem constants (hardcoded per contract)
B, S, H, A = 65536, 128, 256, 18
NCORES = 8
BC = B // NCORES          # 8192 rows per core
CH = 256                  # batch columns per compute chunk
NCH = BC // CH            # 32 chunks
GR = BC // 128            # 64 groups of 128 rows per core
LOADCOLS = 1024           # x DMA tile columns
NLOAD = BC // LOADCOLS
QW = 64                   # Q0 padded action dim (partitions 0..63)
QOFF = 64                 # Qn partition offset (col group 2)
QN_W = 32                 # Qn padded action dim
STK = 96                  # stacked Q partitions per chunk
DONE = 666.0
DISC = 0.9

MM_DT = os.environ.get("BELLMAN_MM_DT", "bf16")  # "bf16" | "f32r"
EP_LIMIT = int(os.environ.get("BELLMAN_EP", "99"))

if MM_DT == "bf16":
    import ml_dtypes
    NP_MDT = ml_dtypes.bfloat16
    TILE_MDT = mybir.dt.bfloat16
else:
    NP_MDT = np.float32
    TILE_MDT = mybir.dt.float32r

F32 = mybir.dt.float32
I32 = mybir.dt.int32
AF = mybir.ActivationFunctionType
OP = mybir.AluOpType
AX = mybir.AxisListType


def _build_program():
    nc = bacc.Bacc("TRN2", target_bir_lowering=False, debug=False)

    x0t = nc.dram_tensor("x0t", [128, BC], TILE_MDT, kind="ExternalInput").ap()
    x1t = nc.dram_tensor("x1t", [128, BC], TILE_MDT, kind="ExternalInput").ap()
    actb = nc.dram_tensor("actb", [128, GR * A], I32, kind="ExternalInput").ap()
    rewb = nc.dram_tensor("rewb", [128, GR], F32, kind="ExternalInput").ap()
    s1b = nc.dram_tensor("s1b", [128, GR], F32, kind="ExternalInput").ap()
    w1 = nc.dram_tensor("w1", [S, H], TILE_MDT, kind="ExternalInput").ap()
    w2 = nc.dram_tensor("w2", [H, H], TILE_MDT, kind="ExternalInput").ap()
    w3p = nc.dram_tensor("w3p", [H, QW], TILE_MDT, kind="ExternalInput").ap()
    b1d = nc.dram_tensor("b1d", [128, 2], F32, kind="ExternalInput").ap()
    b2d = nc.dram_tensor("b2d", [128, 2], F32, kind="ExternalInput").ap()
    b3d = nc.dram_tensor("b3d", [STK, 1], F32, kind="ExternalInput").ap()
    iotad = nc.dram_tensor("iotad", [128, A], F32, kind="ExternalInput").ap()
    identd = nc.dram_tensor("identd", [STK, STK], TILE_MDT, kind="ExternalInput").ap()
    outp = nc.dram_tensor("outp", [128, 1], F32, kind="ExternalOutput").ap()

    from contextlib import ExitStack

    with tile.TileContext(nc) as tc, ExitStack() as ctx:
        singles = ctx.enter_context(tc.tile_pool(name="singles", bufs=1))
        xpool = ctx.enter_context(tc.tile_pool(name="xpool", bufs=2))
        hpool = ctx.enter_context(tc.tile_pool(name="hpool", bufs=2))
        qspool = ctx.enter_context(tc.tile_pool(name="qspool", bufs=2))
        big = ctx.enter_context(tc.tile_pool(name="big", bufs=1))
        ps_h1 = ctx.enter_context(tc.tile_pool(name="ps_h1", bufs=2, space="PSUM"))
        ps_h2 = ctx.enter_context(tc.tile_pool(name="ps_h2", bufs=2, space="PSUM"))
        ps_qt = ctx.enter_context(tc.tile_pool(name="ps_qt", bufs=2, space="PSUM"))
        ps_tp = ctx.enter_context(tc.tile_pool(name="ps_tp", bufs=2, space="PSUM"))

        # --- constants / per-core staging loads ---
        w1_s = singles.tile([S, H], TILE_MDT)
        nc.scalar.dma_start(out=w1_s, in_=w1)
        w2_s = []
        for k in range(2):
            t = singles.tile([128, H], TILE_MDT, tag=f"w2_{k}")
            nc.scalar.dma_start(out=t, in_=w2[k * 128:(k + 1) * 128, :])
            w2_s.append(t)
        w3_s = []
        for k in range(2):
            t = singles.tile([128, QW], TILE_MDT, tag=f"w3_{k}")
            nc.scalar.dma_start(out=t, in_=w3p[k * 128:(k + 1) * 128, :])
            w3_s.append(t)
        b1_s = singles.tile([128, 2], F32, tag="b1")
        nc.scalar.dma_start(out=b1_s, in_=b1d)
        b2_s = singles.tile([128, 2], F32, tag="b2")
        nc.scalar.dma_start(out=b2_s, in_=b2d)
        b3_s = singles.tile([STK, 1], F32, tag="b3")
        nc.scalar.dma_start(out=b3_s, in_=b3d)
        iota_s = singles.tile([128, A], F32, tag="iota")
        nc.scalar.dma_start(out=iota_s, in_=iotad)
        actb_s = singles.tile([128, GR * A], I32, tag="actb")
        rewb_s = singles.tile([128, GR], F32, tag="rewb")
        s1b_s = singles.tile([128, GR], F32, tag="s1b")
        ident = singles.tile([STK, STK], TILE_MDT, tag="ident")
        nc.scalar.dma_start(out=ident, in_=identd)

        # batch-major Q staging: group g (one 128-row slab) occupies cols
        # [96g, 96g+96): Q0 at +0..17, Qn at +64..81.
        qbuf = big.tile([128, GR * STK], TILE_MDT, tag="qbuf")

        relu_idx = [0]

        def relu_copy(dst, src, bias_ap):
            # split relu copies ~17:15 ACT:DVE to equalize engine time
            i = relu_idx[0] % 32
            relu_idx[0] += 1
            if i % 2 == 0 or i == 1:
                nc.scalar.activation(dst, src, AF.Relu, bias=bias_ap, scale=1.0)
            else:
                nc.vector.tensor_scalar(dst, src, bias_ap, 0.0, OP.add, OP.max)

        # ---- software-pipelined main loop ----
        # tick = one chunk-pass (64 ticks). Stage shifts keep every engine's
        # in-order queue free of waits on freshly produced cross-engine data:
        #   t:   mm1[t]          (PE)
        #   t+1: relu1[t]        (ACT/DVE)
        #   t+2: mm2[t]          (PE)
        #   t+3: relu2[t]        (ACT/DVE)
        #   t+4 (odd passes): mm3[chunk] both passes, col-group paired (PE)
        #   t+5: stack copy, t+6: transposes, t+7: qbuf copy
        T = 2 * NCH
        PASS_PER_LOAD = 2 * LOADCOLS // CH
        xL_tiles = {}
        h1p_t, h1s_t, h2p_t, h2s_t = {}, {}, {}, {}
        qt_c, qts_c, tp_c = {}, {}, {}

        # epilogue tiles (allocated up front; ops emitted inline)
        import itertools
        _ep_count = itertools.count(1)

        def _ep():
            return next(_ep_count) <= EP_LIMIT

        actf = big.tile([128, GR * A], F32, tag="actf")
        score = big.tile([128, GR * A], F32, tag="score")
        rowmax = big.tile([128, GR], F32, tag="rowmax")
        onehot = big.tile([128, GR * A], F32, tag="onehot")
        prod = big.tile([128, GR * A], F32, tag="prod")
        q0sel = big.tile([128, GR], F32, tag="q0sel")
        maxqn = big.tile([128, GR], F32, tag="maxqn")
        donem = big.tile([128, GR], F32, tag="donem")
        fac = big.tile([128, GR], F32, tag="fac")
        t1 = big.tile([128, GR], F32, tag="t1")
        t2 = big.tile([128, GR], F32, tag="t2")
        diff = big.tile([128, GR], F32, tag="diff")
        sq = big.tile([128, GR], F32, tag="sq")
        acc = big.tile([128, 1], F32, tag="acc")
        if EP_LIMIT < 99:
            nc.vector.memset(acc, 0.0)

        q3 = qbuf[:, :].rearrange("p (g s) -> p g s", s=STK)
        a3 = lambda t_: t_[:, :].rearrange("p (g a) -> p g a", a=A)
        HG = GR // 4

        def ep_front(hh):
            # argmax/onehot of actions: independent of the MLP, runs early
            gsl = slice(hh * HG, (hh + 1) * HG)
            asl = slice(hh * HG * A, (hh + 1) * HG * A)
            iot_b = iota_s[:, None, :].broadcast_to([128, HG, A])
            if _ep():
                nc.scalar.activation(actf[:, asl], actb_s[:, asl],
                                     AF.Copy, scale=32.0)
            if _ep():
                nc.gpsimd.tensor_tensor(a3(score)[:, gsl], a3(actf)[:, gsl],
                                        iot_b, OP.subtract)
            if _ep():
                nc.vector.tensor_reduce(rowmax[:, gsl], a3(score)[:, gsl],
                                        AX.X, OP.max)
            if _ep():
                nc.vector.tensor_tensor(
                    a3(onehot)[:, gsl], a3(score)[:, gsl],
                    rowmax[:, gsl, None].broadcast_to([128, HG, A]),
                    OP.is_equal)
            if _ep():
                nc.vector.tensor_scalar(donem[:, gsl], s1b_s[:, gsl],
                                        DONE, None, OP.is_equal)
            if _ep():
                nc.vector.tensor_scalar(fac[:, gsl], donem[:, gsl],
                                        -DISC, DISC, OP.mult, OP.add)

        def ep_tail(hh):
            # needs qbuf for groups in the half
            gsl = slice(hh * HG, (hh + 1) * HG)
            if _ep():
                nc.gpsimd.tensor_tensor(a3(prod)[:, gsl], a3(onehot)[:, gsl],
                                        q3[:, gsl, 0:A], OP.mult)
            if _ep():
                nc.vector.tensor_reduce(q0sel[:, gsl], a3(prod)[:, gsl],
                                        AX.X, OP.add)
            if _ep():
                nc.vector.tensor_reduce(maxqn[:, gsl], q3[:, gsl, QOFF:QOFF + A],
                                        AX.X, OP.max)
            if _ep():
                nc.vector.tensor_tensor(t1[:, gsl], maxqn[:, gsl], fac[:, gsl],
                                        OP.mult)
            if _ep():
                nc.vector.tensor_tensor(t2[:, gsl], t1[:, gsl], rewb_s[:, gsl],
                                        OP.add)
            if _ep():
                nc.vector.tensor_tensor(diff[:, gsl], q0sel[:, gsl], t2[:, gsl],
                                        OP.subtract)
            if _ep():
                nc.vector.tensor_tensor(sq[:, gsl], diff[:, gsl], diff[:, gsl],
                                        OP.mult)

        def do_dma(li):
            x0L = xpool.tile([128, LOADCOLS], TILE_MDT, tag="x0")
            x1L = xpool.tile([128, LOADCOLS], TILE_MDT, tag="x1")
            nc.sync.dma_start(out=x0L,
                              in_=x0t[:, li * LOADCOLS:(li + 1) * LOADCOLS])
            nc.sync.dma_start(out=x1L,
                              in_=x1t[:, li * LOADCOLS:(li + 1) * LOADCOLS])
            xL_tiles[li] = (x0L, x1L)

        def xs_for(t):
            c, pa = t // 2, t % 2
            li = (c * CH) // LOADCOLS
            ci = (c * CH) % LOADCOLS // CH
            return xL_tiles[li][pa][:, ci * CH:(ci + 1) * CH]

        def st_mm1(t):
            h1p = ps_h1.tile([128, 2, CH], F32, tag="h1p", name=f"h1p_{t}")
            xs = xs_for(t)
            for m in range(2):
                nc.tensor.matmul(h1p[:, m, :], w1_s[:, m * 128:(m + 1) * 128],
                                 xs, start=True, stop=True)
            h1p_t[t] = h1p

        def st_relu1(t):
            h1s = hpool.tile([128, 2, CH], TILE_MDT, tag="h1s", bufs=3,
                             name=f"h1s_{t}")
            relu_copy(h1s[:, :, :].rearrange("p a b -> p (a b)"),
                      h1p_t.pop(t)[:, :, :].rearrange("p a b -> p (a b)"),
                      b1_s[:, 0:1])
            h1s_t[t] = h1s

        def st_mm2(t):
            h2p = ps_h2.tile([128, 2, CH], F32, tag="h2p", name=f"h2p_{t}")
            h1s = h1s_t.pop(t)
            for m in range(2):
                for k in range(2):
                    nc.tensor.matmul(h2p[:, m, :],
                                     w2_s[k][:, m * 128:(m + 1) * 128],
                                     h1s[:, k, :], start=(k == 0), stop=(k == 1))
            h2p_t[t] = h2p

        def st_relu2(t):
            h2s = hpool.tile([128, 2, CH], TILE_MDT, tag="h2s", bufs=4,
                             name=f"h2s_{t}")
            relu_copy(h2s[:, :, :].rearrange("p a b -> p (a b)"),
                      h2p_t.pop(t)[:, :, :].rearrange("p a b -> p (a b)"),
                      b2_s[:, 0:1])
            h2s_t[t] = h2s

        def st_mm3(c):
            # both passes, Q0 on col group 0/1, Qn on col group 2 (concurrent)
            qt_ps = ps_qt.tile([STK, CH], F32, tag="qt", name=f"qt_{c}")
            h2s0 = h2s_t.pop(2 * c)
            h2s1 = h2s_t.pop(2 * c + 1)
            for k in range(2):
                nc.tensor.matmul(qt_ps[0:QW, :], w3_s[k], h2s0[:, k, :],
                                 start=(k == 0), stop=(k == 1))
            for k in range(2):
                nc.tensor.matmul(qt_ps[QOFF:QOFF + QN_W, :], w3_s[k][:, 0:QN_W],
                                 h2s1[:, k, :], start=(k == 0), stop=(k == 1),
                                 tile_position=(0, QOFF))
            qt_c[c] = qt_ps

        def st_stack(c):
            qts = qspool.tile([STK, CH], TILE_MDT, tag="qts", name=f"qts_{c}")
            nc.scalar.activation(qts, qt_c.pop(c), AF.Identity, bias=b3_s,
                                 scale=1.0)
            qts_c[c] = qts

        def st_tp(c):
            tp_ps = ps_tp.tile([128, 2, STK], TILE_MDT, tag="tp", name=f"tp_{c}")
            qts = qts_c.pop(c)
            for j in range(2):
                nc.tensor.transpose(tp_ps[:, j, :],
                                    qts[:, j * 128:(j + 1) * 128], ident)
            tp_c[c] = tp_ps

        def st_qb(c):
            nc.vector.tensor_copy(
                qbuf[:, c * 2 * STK:(c + 1) * 2 * STK],
                tp_c.pop(c)[:, :, :].rearrange("p a b -> p (a b)"))

        do_dma(0)
        NQ = 4  # epilogue emitted in quarters
        tails_done = 0
        for t in range(T + 8):
            # prefetch next x load 4 passes early
            nt = t + 4
            if nt < T and nt % PASS_PER_LOAD == 0:
                do_dma(nt // PASS_PER_LOAD)
            if t == 2:
                nc.scalar.dma_start(out=actb_s, in_=actb)
                nc.scalar.dma_start(out=rewb_s, in_=rewb)
                nc.scalar.dma_start(out=s1b_s, in_=s1b)
            if t == 10:
                for hh in range(NQ):
                    ep_front(hh)
            if t < T:
                st_mm1(t)
            if 0 <= t - 1 < T:
                st_relu1(t - 1)
            if 0 <= t - 2 < T:
                st_mm2(t - 2)
            if 0 <= t - 3 < T:
                st_relu2(t - 3)
            if 0 <= t - 4 < T and (t - 4) % 2 == 1:
                st_mm3((t - 4) // 2)
            if 0 <= t - 5 < T and (t - 5) % 2 == 1:
                st_stack((t - 5) // 2)
            if 0 <= t - 6 < T and (t - 6) % 2 == 1:
                st_tp((t - 6) // 2)
            if 0 <= t - 7 < T and (t - 7) % 2 == 1:
                c = (t - 7) // 2
                st_qb(c)
                while tails_done < NQ - 1 and c + 1 >= (tails_done + 1) * (NCH // NQ):
                    ep_tail(tails_done)
                    tails_done += 1
        while tails_done < NQ:
            ep_tail(tails_done)
            tails_done += 1
        if _ep():
            nc.vector.tensor_reduce(acc, sq, AX.X, OP.add)
        nc.sync.dma_start(out=outp, in_=acc)

    nc.compile()
    return nc


_CACHE = {}


def _get_program():
    if "nc" not in _CACHE:
        _CACHE["nc"] = _build_program()
    return _CACHE["nc"]


def _prep_in_maps(inputs):
    st0 = np.asarray(inputs["states0"], dtype=np.float32)
    st1 = np.asarray(inputs["states1"], dtype=np.float32)
    act = np.asarray(inputs["actions"], dtype=np.int32)
    rew = np.asarray(inputs["rewards"], dtype=np.float32)
    W1 = np.asarray(inputs["W1"], dtype=np.float32).astype(NP_MDT)
    W2 = np.asarray(inputs["W2"], dtype=np.float32).astype(NP_MDT)
    W3 = np.asarray(inputs["W3"], dtype=np.float32)
    b1 = np.asarray(inputs["b1"], dtype=np.float32)
    b2 = np.asarray(inputs["b2"], dtype=np.float32)
    b3 = np.asarray(inputs["b3"], dtype=np.float32)

    w3pad = np.zeros((H, QW), np.float32)
    w3pad[:, :A] = W3
    w3pad = w3pad.astype(NP_MDT)
    b1m = np.ascontiguousarray(b1.reshape(2, 128).T)
    b2m = np.ascontiguousarray(b2.reshape(2, 128).T)
    b3p = np.zeros((STK, 1), np.float32)
    b3p[0:A, 0] = b3
    b3p[QOFF:QOFF + A, 0] = b3
    iota = np.ascontiguousarray(
        np.broadcast_to(np.arange(A, dtype=np.float32), (128, A)))
    ident = np.eye(STK, dtype=np.float32).astype(NP_MDT)

    in_maps = []
    for c in range(NCORES):
        r0, r1 = c * BC, (c + 1) * BC
        in_maps.append({
            "x0t": np.ascontiguousarray(st0[r0:r1].T).astype(NP_MDT),
            "x1t": np.ascontiguousarray(st1[r0:r1].T).astype(NP_MDT),
            "actb": np.ascontiguousarray(
                act[r0:r1].reshape(GR, 128, A).transpose(1, 0, 2).reshape(128, GR * A)),
            "rewb": np.ascontiguousarray(rew[r0:r1].reshape(GR, 128).T),
            "s1b": np.ascontiguousarray(st1[r0:r1, 0].reshape(GR, 128).T),
            "w1": W1, "w2": W2, "w3p": w3pad,
            "b1d": b1m, "b2d": b2m, "b3d": b3p, "iotad": iota,
            "identd": ident,
        })
    return in_maps


def _run(inputs, trace=False):
    nc = _get_program()
    in_maps = _prep_in_maps(inputs)
    res = run_bass_kernel_spmd(nc, in_maps, core_ids=list(range(NCORES)),
                               trace=trace)
    total = 0.0
    for r in res.results:
        total += float(np.asarray(r["outp"], dtype=np.float64).sum())
    return np.array(np.float32(total)), res


def kernel(**inputs) -> np.ndarray:
    val, _ = _run(inputs, trace=False)
    return val



# revision 8
# speedup vs baseline: 1.0382x; 1.0382x over previous
"""Bass/Trainium2 kernel for nn_BellmanLoss (8-core data-parallel).

Math: the reference's scatter makes Q_new differ from Q0 only at
a_i = argmax_j(actions[i, j]) (first max), so

    loss = sum_i (Q0[i, a_i] - target_i)^2
    target_i = r_i + 0.9 * max_a Qn[i, a] * (1 - done_i),  done_i = (states1[i,0] == 666)

Per core: 8192 rows, CH=512 batch columns per tick, 32 ticks (even=state0,
odd=state1 chunks). MLP runs feature-major (h^T = [features, batch]):
  mm1: fp8 non-DR (K=128), N=512  -> h1p PSUM [128,2,512]
  relu1: ACT/DVE copy PSUM->SBUF fp8 (+b1)
  mm2: fp8 DoubleRow (K=256 packed), N=512 -> h2p PSUM [128,512] per m
  relu2: per-m copies -> h2s fp8
  mm3: fp8 DoubleRow, Q^T [18,512] packed 4x32-partition groups per qt bank
  stack: PSUM->SBUF bf16 (+b3)
  dma_start_transpose: qs [128,512] -> qbuf [128,4,128] batch-major
Epilogue (batch-major, small FD): argmax-onehot select of Q0, max of Qn,
target, per-partition loss partials. Host does layout-only prep and the
final 1024-element sum.
"""

import os
import numpy as np
import ml_dtypes

import concourse.bass as bass
import concourse.mybir as mybir
import concourse.tile as tile
from concourse import bacc
from concourse.bass_utils import run_bass_kernel_spmd

# Problem constants (hardcoded per contract)
B, S, H, A = 65536, 128, 256, 18
NCORES = 8
BC = B // NCORES          # 8192 rows per core
CH = 512                  # batch columns per tick
T = 2 * (BC // CH)        # 32 ticks (x0/x1 interleaved)
NQ = BC // CH // 2        # 8 qt tiles (each: 2 chunk-pairs x (Q0,Qn))
GR = BC // 128            # 64 batch blocks of 128 rows
LOADCOLS = 2048           # x DMA tile columns
DONE = 666.0
DISC = 0.9

FP8 = mybir.dt.float8e4
BF16 = mybir.dt.bfloat16
FP16 = mybir.dt.float16
F32 = mybir.dt.float32
I8 = mybir.dt.int8
AF = mybir.ActivationFunctionType
OP = mybir.AluOpType
AX = mybir.AxisListType
DR = mybir.MatmulPerfMode.DoubleRow

NP_FP8 = ml_dtypes.float8_e4m3
NP_BF16 = ml_dtypes.bfloat16

USE_DR = os.environ.get("BELLMAN_DR", "1") == "1"
USE_DMAT = os.environ.get("BELLMAN_DMAT", "1") == "1"


def _build_program():
    nc = bacc.Bacc("TRN2", target_bir_lowering=False, debug=False)

    x0t = nc.dram_tensor("x0t", [128, BC], FP8, kind="ExternalInput").ap()
    x1t = nc.dram_tensor("x1t", [128, BC], FP8, kind="ExternalInput").ap()
    actb = nc.dram_tensor("actb", [128, GR * A], I8, kind="ExternalInput").ap()
    rewb = nc.dram_tensor("rewb", [128, GR], F32, kind="ExternalInput").ap()
    s1b = nc.dram_tensor("s1b", [128, GR], F32, kind="ExternalInput").ap()
    w1 = nc.dram_tensor("w1", [S, H], FP8, kind="ExternalInput").ap()
    w2km = nc.dram_tensor("w2km", [128, 2 * H], FP8, kind="ExternalInput").ap()
    w3s = nc.dram_tensor("w3s", [128, 2 * 32], FP8, kind="ExternalInput").ap()
    b1d = nc.dram_tensor("b1d", [128, 2], F32, kind="ExternalInput").ap()
    b2d = nc.dram_tensor("b2d", [128, 2], F32, kind="ExternalInput").ap()
    b3st = nc.dram_tensor("b3st", [128, 1], F32, kind="ExternalInput").ap()
    iotad = nc.dram_tensor("iotad", [128, A], FP16, kind="ExternalInput").ap()
    outp = nc.dram_tensor("outp", [128, 1], F32, kind="ExternalOutput").ap()

    from contextlib import ExitStack

    with tile.TileContext(nc) as tc, ExitStack() as ctx:
        singles = ctx.enter_context(tc.tile_pool(name="singles", bufs=1))
        xpool = ctx.enter_context(tc.tile_pool(name="xpool", bufs=2))
        h1spool = ctx.enter_context(tc.tile_pool(name="h1s", bufs=3))
        h2spool = ctx.enter_context(tc.tile_pool(name="h2s", bufs=4))
        big = ctx.enter_context(tc.tile_pool(name="big", bufs=1))
        ps_h1 = ctx.enter_context(tc.tile_pool(name="ps_h1", bufs=2, space="PSUM"))
        ps_h2 = ctx.enter_context(tc.tile_pool(name="ps_h2", bufs=3, space="PSUM"))
        ps_qt = ctx.enter_context(tc.tile_pool(name="ps_qt", bufs=1, space="PSUM"))

        # --- constants / per-core staging loads (scalar queue, early) ---
        w1_s = singles.tile([S, H], FP8)
        nc.scalar.dma_start(out=w1_s, in_=w1)
        w2_s = singles.tile([128, 2, H], FP8, tag="w2")
        nc.scalar.dma_start(
            out=w2_s[:, :, :].rearrange("p a b -> p (a b)"), in_=w2km)
        w3_s = singles.tile([128, 2, 32], FP8, tag="w3")
        nc.scalar.dma_start(
            out=w3_s[:, :, :].rearrange("p a b -> p (a b)"), in_=w3s)
        b1_s = singles.tile([128, 2], F32, tag="b1")
        nc.scalar.dma_start(out=b1_s, in_=b1d)
        b2_s = singles.tile([128, 2], F32, tag="b2")
        nc.scalar.dma_start(out=b2_s, in_=b2d)
        b3_s = singles.tile([128, 1], F32, tag="b3")
        nc.scalar.dma_start(out=b3_s, in_=b3st)
        iota_s = singles.tile([128, A], FP16, tag="iota")
        nc.scalar.dma_start(out=iota_s, in_=iotad)
        actb_s = singles.tile([128, GR * A], I8, tag="actb")
        rewb_s = singles.tile([128, GR], F32, tag="rewb")
        s1b_s = singles.tile([128, GR], F32, tag="s1b")

        # qs: stacked Q^T in SBUF bf16 (stack copies write, dma-transpose reads)
        qs = big.tile([128, NQ, CH], BF16, tag="qs")
        # qbuf: batch-major Q (partition = batch-within-128-block)
        qbuf = big.tile([128, NQ, 4, 128], BF16, tag="qbuf")

        # epilogue tiles
        actf = big.tile([128, GR * A], FP16, tag="actf")
        score = big.tile([128, GR, A], FP16, tag="score")
        rowmax = big.tile([128, GR], FP16, tag="rowmax")
        onehot = big.tile([128, GR, A], BF16, tag="onehot")
        donem = big.tile([128, GR], F32, tag="donem")
        fac = big.tile([128, GR], F32, tag="fac")
        prod = big.tile([128, GR, A], BF16, tag="prod")
        q0sel = big.tile([128, GR], F32, tag="q0sel")
        maxqn = big.tile([128, GR], F32, tag="maxqn")
        t1 = big.tile([128, GR], F32, tag="t1")
        t2 = big.tile([128, GR], F32, tag="t2")
        diff = big.tile([128, GR], F32, tag="diff")
        sq = big.tile([128, GR], F32, tag="sq")
        acc = big.tile([128, 1], F32, tag="acc")

        xL = {}
        h1p_t, h1s_t, h2pa_t, h2pb_t, h2s_t, qt_q = {}, {}, {}, {}, {}, {}

        def do_dma(li):
            x0L = xpool.tile([128, LOADCOLS], FP8, tag="x0")
            x1L = xpool.tile([128, LOADCOLS], FP8, tag="x1")
            nc.sync.dma_start(out=x0L,
                              in_=x0t[:, li * LOADCOLS:(li + 1) * LOADCOLS])
            nc.sync.dma_start(out=x1L,
                              in_=x1t[:, li * LOADCOLS:(li + 1) * LOADCOLS])
            xL[li] = (x0L, x1L)

        def xs_for(t):
            c, pa = t // 2, t % 2
            li = (c * CH) // LOADCOLS
            ci = (c * CH) % LOADCOLS // CH
            return xL[li][pa][:, ci * CH:(ci + 1) * CH]

        def st_mm1(t):
            h1p = ps_h1.tile([128, 2, CH], F32, tag="h1p", name=f"h1p_{t}")
            xs = xs_for(t)
            for m in range(2):
                nc.tensor.matmul(h1p[:, m, :], w1_s[:, m * 128:(m + 1) * 128],
                                 xs, start=True, stop=True)
            h1p_t[t] = h1p

        # relu copy engine alternation
        def relu_engine(idx):
            return nc.scalar if idx % 2 == 0 else nc.vector

        def emit_relu(eng, dst, src, bias_ap):
            if eng is nc.scalar:
                nc.scalar.activation(dst, src, AF.Relu, bias=bias_ap, scale=1.0)
            else:
                nc.vector.tensor_scalar(dst, src, bias_ap, 0.0, OP.add, OP.max)

        def st_relu1(t):
            h1s = h1spool.tile([128, 2, CH], FP8, tag="h1s", name=f"h1s_{t}")
            emit_relu(relu_engine(t),
                      h1s[:, :, :].rearrange("p a b -> p (a b)"),
                      h1p_t.pop(t)[:, :, :].rearrange("p a b -> p (a b)"),
                      b1_s[:, 0:1])
            h1s_t[t] = h1s

        def st_mm2(t):
            h1s = h1s_t.pop(t)
            if USE_DR:
                h2pa = ps_h2.tile([128, CH], F32, tag="h2p", name=f"h2pa_{t}")
                nc.tensor.matmul(h2pa, w2_s[:, :, 0:128], h1s[:, :, :],
                                 start=True, stop=True, perf_mode=DR)
                h2pb = ps_h2.tile([128, CH], F32, tag="h2p", name=f"h2pb_{t}")
                nc.tensor.matmul(h2pb, w2_s[:, :, 128:256], h1s[:, :, :],
                                 start=True, stop=True, perf_mode=DR)
            else:
                h2pa = ps_h2.tile([128, CH], F32, tag="h2p", name=f"h2pa_{t}")
                for k in range(2):
                    nc.tensor.matmul(h2pa, w2_s[:, k, 0:128], h1s[:, k, :],
                                     start=(k == 0), stop=(k == 1))
                h2pb = ps_h2.tile([128, CH], F32, tag="h2p", name=f"h2pb_{t}")
                for k in range(2):
                    nc.tensor.matmul(h2pb, w2_s[:, k, 128:256], h1s[:, k, :],
                                     start=(k == 0), stop=(k == 1))
            h2pa_t[t] = h2pa
            h2pb_t[t] = h2pb

        def st_relu2(t):
            h2s = h2spool.tile([128, 2, CH], FP8, tag="h2s", name=f"h2s_{t}")
            # m0 on engine (t+1)%2, m1 on engine t%2 (anti-collide with relu1)
            emit_relu(relu_engine(t + 1), h2s[:, 0, :], h2pa_t.pop(t),
                      b2_s[:, 0:1])
            emit_relu(relu_engine(t), h2s[:, 1, :], h2pb_t.pop(t),
                      b2_s[:, 1:2])
            h2s_t[t] = h2s

        def st_mm3(c):
            # chunk-pair c: Q0 from h2s[2c] (state0), Qn from h2s[2c+1]
            q = c // 2
            gp = (c % 2) * 2  # group pair base (0 or 2)
            if q not in qt_q:
                qt_q[q] = ps_qt.tile([128, CH], F32, tag="qt", name=f"qt_{q}")
            qt = qt_q[q]
            h2s0 = h2s_t.pop(2 * c)
            h2s1 = h2s_t.pop(2 * c + 1)
            # col-tiled: Q0 and Qn MMs run concurrently on different 32-col
            # groups of the PE array (DoubleRow is illegal off col-group 0)
            for k in range(2):
                for gi, h2sx in ((gp, h2s0), (gp + 1, h2s1)):
                    po = gi * 32
                    nc.tensor.matmul(qt[po:po + A, :], w3_s[:, k, 0:A],
                                     h2sx[:, k, :], start=(k == 0),
                                     stop=(k == 1), tile_position=(0, po))

        def st_stack(q):
            # PSUM f32 -> SBUF bf16 with b3 bias (per stacked partition)
            eng = relu_engine(q)
            if eng is nc.scalar:
                nc.scalar.activation(qs[:, q, :], qt_q[q], AF.Identity,
                                     bias=b3_s[:, 0:1], scale=1.0)
            else:
                nc.vector.tensor_scalar(qs[:, q, :], qt_q[q], b3_s[:, 0:1],
                                        None, OP.add)
            qt_q.pop(q)

        def st_dmaT(q):
            nc.sync.dma_start_transpose(out=qbuf[:, q, :, :], in_=qs[:, q, :])

        def ep_front():
            # argmax/onehot of actions; done mask; discount factor
            a3 = actf[:, :].rearrange("p (g a) -> p g a", a=A)
            iot_b = iota_s[:, None, :].broadcast_to([128, GR, A])
            nc.gpsimd.tensor_scalar(actf, actb_s, 32.0, None, OP.mult)
            nc.gpsimd.tensor_tensor(score, a3, iot_b, OP.subtract)
            nc.vector.tensor_reduce(rowmax, score, AX.X, OP.max)
            nc.vector.tensor_tensor(
                onehot, score, rowmax[:, :, None].broadcast_to([128, GR, A]),
                OP.is_equal)
            nc.vector.tensor_scalar(donem, s1b_s, DONE, None, OP.is_equal)
            nc.vector.tensor_scalar(fac, donem, -DISC, DISC, OP.mult, OP.add)

        def ep_tail(q):
            # per qt tile: 8 batch blocks (g' = q*8 + w*2 + pair)
            qb = qbuf[:, q, :, :].rearrange("p w (g s) -> p w g s", s=32)
            gsl = slice(q * 8, (q + 1) * 8)
            for pair in range(2):
                # blocks g' = q*8 + w*2 + pair, w = 0..3
                oh = onehot[:, q * 8 + pair:(q + 1) * 8:2, :]   # [128,4,A]
                q0 = qb[:, :, 2 * pair, 0:A]                     # [128,4,A]
                qn = qb[:, :, 2 * pair + 1, 0:A]
                pr = prod[:, q * 8 + pair:(q + 1) * 8:2, :]
                nc.vector.tensor_tensor(pr, oh, q0, OP.mult)
                nc.vector.tensor_reduce(
                    q0sel[:, q * 8 + pair:(q + 1) * 8:2], pr, AX.X, OP.add)
                nc.vector.tensor_reduce(
                    maxqn[:, q * 8 + pair:(q + 1) * 8:2], qn, AX.X, OP.max)
            nc.vector.tensor_tensor(t1[:, gsl], maxqn[:, gsl], fac[:, gsl],
                                    OP.mult)
            nc.vector.tensor_tensor(t2[:, gsl], t1[:, gsl], rewb_s[:, gsl],
                                    OP.add)
            nc.vector.tensor_tensor(diff[:, gsl], q0sel[:, gsl], t2[:, gsl],
                                    OP.subtract)
            nc.vector.tensor_tensor(sq[:, gsl], diff[:, gsl], diff[:, gsl],
                                    OP.mult)

        # ---- main software-pipelined loop ----
        do_dma(0)
        PASS_PER_LOAD = 2 * LOADCOLS // CH   # ticks covered per load pair
        tails = 0
        for t in range(T + 10):
            nt = t + 4
            if nt < T and nt % PASS_PER_LOAD == 0:
                do_dma(nt // PASS_PER_LOAD)
            if t == 2:
                nc.scalar.dma_start(out=actb_s, in_=actb)
                nc.scalar.dma_start(out=rewb_s, in_=rewb)
                nc.scalar.dma_start(out=s1b_s, in_=s1b)
            if t == 8:
                ep_front()
            if t < T:
                st_mm1(t)
            if 0 <= t - 2 < T:
                st_mm2(t - 2)
            if 0 <= t - 5 and (t - 5) % 2 == 0 and (t - 5) // 2 < T // 2:
                st_mm3((t - 5) // 2)
            if 0 <= t - 3 < T:
                st_relu2(t - 3)
            if 0 <= t - 1 < T:
                st_relu1(t - 1)
            if t >= 8 and (t - 8) % 4 == 0:
                q = (t - 8) // 4
                if q < NQ:
                    st_stack(q)
            if t >= 9 and (t - 9) % 4 == 0:
                q = (t - 9) // 4
                if q < NQ:
                    st_dmaT(q)
            if t >= 11 and (t - 11) % 4 == 0:
                q = (t - 11) // 4
                if q < NQ:
                    ep_tail(q)
                    tails += 1
        while tails < NQ:
            ep_tail(tails)
            tails += 1
        nc.vector.tensor_reduce(acc, sq, AX.X, OP.add)
        nc.sync.dma_start(out=outp, in_=acc)

    nc.compile()
    return nc


_CACHE = {}


def _get_program():
    if "nc" not in _CACHE:
        _CACHE["nc"] = _build_program()
    return _CACHE["nc"]


def _block_perm():
    # qbuf block order g' -> source batch block b
    perm = np.empty(GR, np.int64)
    for gp in range(GR):
        q, r = divmod(gp, 8)
        w, pair = divmod(r, 2)
        perm[gp] = (2 * q + pair) * 4 + w
    return perm


def _prep_in_maps(inputs):
    st0 = np.asarray(inputs["states0"], dtype=np.float32)
    st1 = np.asarray(inputs["states1"], dtype=np.float32)
    act = np.asarray(inputs["actions"], dtype=np.int32)
    rew = np.asarray(inputs["rewards"], dtype=np.float32)
    W1 = np.asarray(inputs["W1"], dtype=np.float32)
    W2 = np.asarray(inputs["W2"], dtype=np.float32)
    W3 = np.asarray(inputs["W3"], dtype=np.float32)
    b1 = np.asarray(inputs["b1"], dtype=np.float32)
    b2 = np.asarray(inputs["b2"], dtype=np.float32)
    b3 = np.asarray(inputs["b3"], dtype=np.float32)

    # sanitize DONE sentinel (666 > fp8e4m3 max); done rows' Qn is masked out
    s1col = st1[:, 0].copy()
    st1f = st1.copy()
    st1f[:, 0] = np.where(s1col == DONE, 0.0, s1col)

    w1f = W1.astype(NP_FP8)
    w2km = np.ascontiguousarray(
        W2.reshape(2, 128, H).transpose(1, 0, 2)).astype(NP_FP8).reshape(128, 2 * H)
    w3p = np.zeros((128, 2, 32), np.float32)
    w3p[:, :, :A] = W3.reshape(2, 128, A).transpose(1, 0, 2)
    w3s = w3p.astype(NP_FP8).reshape(128, 2 * 32)
    b1m = np.ascontiguousarray(b1.reshape(2, 128).T)
    b2m = np.ascontiguousarray(b2.reshape(2, 128).T)
    b3stk = np.zeros((128, 1), np.float32)
    for g in range(4):
        b3stk[g * 32:g * 32 + A, 0] = b3
    iota = np.ascontiguousarray(
        np.broadcast_to(np.arange(A, dtype=np.float16), (128, A)))

    perm = _block_perm()
    act8 = act.astype(np.int8)

    in_maps = []
    for c in range(NCORES):
        r0, r1 = c * BC, (c + 1) * BC
        actc = act8[r0:r1].reshape(GR, 128, A)[perm]
        rewc = rew[r0:r1].reshape(GR, 128)[perm]
        s1c = s1col[r0:r1].reshape(GR, 128)[perm]
        in_maps.append({
            "x0t": np.ascontiguousarray(st0[r0:r1].T).astype(NP_FP8),
            "x1t": np.ascontiguousarray(st1f[r0:r1].T).astype(NP_FP8),
            "actb": np.ascontiguousarray(
                actc.transpose(1, 0, 2).reshape(128, GR * A)),
            "rewb": np.ascontiguousarray(rewc.transpose(1, 0)),
            "s1b": np.ascontiguousarray(s1c.transpose(1, 0)),
            "w1": w1f, "w2km": w2km, "w3s": w3s,
            "b1d": b1m, "b2d": b2m, "b3st": b3stk, "iotad": iota,
        })
    return in_maps


def _run(inputs, trace=False):
    nc = _get_program()
    in_maps = _prep_in_maps(inputs)
    res = run_bass_kernel_spmd(nc, in_maps, core_ids=list(range(NCORES)),
                               trace=trace)
    total = 0.0
    for r in res.results:
        total += float(np.asarray(r["outp"], dtype=np.float64).sum())
    return np.array(np.float32(total)), res


def kernel(**inputs) -> np.ndarray:
    val, _ = _run(inputs, trace=False)
    return val


# revision 12
# speedup vs baseline: 1.2611x; 1.2147x over previous
"""Bass/Trainium2 kernel for nn_BellmanLoss (8-core data-parallel).

Math: the reference's scatter makes Q_new differ from Q0 only at
a_i = argmax_j(actions[i, j]) (first max), so

    loss = sum_i (Q0[i, a_i] - target_i)^2
    target_i = r_i + 0.9 * max_a Qn[i, a] * (1 - done_i),  done_i = (states1[i,0] == 666)

Per core: 8192 rows, CH=512 batch columns per tick, 32 ticks (even=state0,
odd=state1 chunks). MLP runs feature-major (h^T = [features, batch]):
  mm1: fp8 non-DR (K=128), N=512  -> h1p PSUM [128,2,512]
  relu1: ACT/DVE copy PSUM->SBUF fp8 (+b1)
  mm2: fp8 DoubleRow (K=256 packed), N=512 -> h2p PSUM [128,512] per m
  relu2: per-m copies -> h2s fp8
  mm3: fp8 DoubleRow, Q^T [18,512] packed 4x32-partition groups per qt bank
  stack: PSUM->SBUF bf16 (+b3)
  dma_start_transpose: qs [128,512] -> qbuf [128,4,128] batch-major
Epilogue (batch-major, small FD): argmax-onehot select of Q0, max of Qn,
target, per-partition loss partials. Host does layout-only prep and the
final 1024-element sum.
"""

import os
import numpy as np
import ml_dtypes

import concourse.bass as bass
import concourse.mybir as mybir
import concourse.tile as tile
from concourse import bacc
from concourse.bass_utils import run_bass_kernel_spmd

# Problem constants (hardcoded per contract)
B, S, H, A = 65536, 128, 256, 18
NCORES = 8
BC = B // NCORES          # 8192 rows per core
CH = 512                  # batch columns per tick
T = 2 * (BC // CH)        # 32 ticks (x0/x1 interleaved)
NQ = BC // CH // 2        # 8 qt tiles (each: 2 chunk-pairs x (Q0,Qn))
GR = BC // 128            # 64 batch blocks of 128 rows
LOADCOLS = 2048           # x DMA tile columns
DONE = 666.0
DISC = 0.9

FP8 = mybir.dt.float8e4
BF16 = mybir.dt.bfloat16
FP16 = mybir.dt.float16
F32 = mybir.dt.float32
I8 = mybir.dt.int8
AF = mybir.ActivationFunctionType
OP = mybir.AluOpType
AX = mybir.AxisListType
DR = mybir.MatmulPerfMode.DoubleRow

NP_FP8 = ml_dtypes.float8_e4m3
NP_BF16 = ml_dtypes.bfloat16

USE_DR = os.environ.get("BELLMAN_DR", "1") == "1"
USE_DMAT = os.environ.get("BELLMAN_DMAT", "1") == "1"


def _build_program():
    nc = bacc.Bacc("TRN2", target_bir_lowering=False, debug=False)

    x0t = nc.dram_tensor("x0t", [128, BC], FP8, kind="ExternalInput").ap()
    x1t = nc.dram_tensor("x1t", [128, BC], FP8, kind="ExternalInput").ap()
    actb = nc.dram_tensor("actb", [128, GR * A], I8, kind="ExternalInput").ap()
    rewb = nc.dram_tensor("rewb", [128, GR], F32, kind="ExternalInput").ap()
    s1b = nc.dram_tensor("s1b", [128, GR], F32, kind="ExternalInput").ap()
    w1 = nc.dram_tensor("w1", [S, H], FP8, kind="ExternalInput").ap()
    w2km = nc.dram_tensor("w2km", [128, 2 * H], FP8, kind="ExternalInput").ap()
    w3s = nc.dram_tensor("w3s", [128, 2 * 32], FP8, kind="ExternalInput").ap()
    b1d = nc.dram_tensor("b1d", [128, 2], F32, kind="ExternalInput").ap()
    b2d = nc.dram_tensor("b2d", [128, 2], F32, kind="ExternalInput").ap()
    b3st = nc.dram_tensor("b3st", [128, 1], F32, kind="ExternalInput").ap()
    iotad = nc.dram_tensor("iotad", [128, A], FP16, kind="ExternalInput").ap()
    outp = nc.dram_tensor("outp", [128, 1], F32, kind="ExternalOutput").ap()

    from contextlib import ExitStack

    with tile.TileContext(nc) as tc, ExitStack() as ctx:
        singles = ctx.enter_context(tc.tile_pool(name="singles", bufs=1))
        xpool = ctx.enter_context(tc.tile_pool(name="xpool", bufs=2))
        h1spool = ctx.enter_context(tc.tile_pool(name="h1s", bufs=3))
        h2spool = ctx.enter_context(tc.tile_pool(name="h2s", bufs=4))
        big = ctx.enter_context(tc.tile_pool(name="big", bufs=1))
        ps_h1 = ctx.enter_context(tc.tile_pool(name="ps_h1", bufs=2, space="PSUM"))
        ps_h2 = ctx.enter_context(tc.tile_pool(name="ps_h2", bufs=3, space="PSUM"))
        ps_qt = ctx.enter_context(tc.tile_pool(name="ps_qt", bufs=1, space="PSUM"))

        # --- constants / per-core staging loads (scalar queue, early) ---
        w1_s = singles.tile([S, H], FP8)
        nc.scalar.dma_start(out=w1_s, in_=w1)
        w2_s = singles.tile([128, 2, H], FP8, tag="w2")
        nc.scalar.dma_start(
            out=w2_s[:, :, :].rearrange("p a b -> p (a b)"), in_=w2km)
        w3_s = singles.tile([128, 2, 32], FP8, tag="w3")
        nc.scalar.dma_start(
            out=w3_s[:, :, :].rearrange("p a b -> p (a b)"), in_=w3s)
        b1_s = singles.tile([128, 2], F32, tag="b1")
        nc.scalar.dma_start(out=b1_s, in_=b1d)
        b2_s = singles.tile([128, 2], F32, tag="b2")
        nc.scalar.dma_start(out=b2_s, in_=b2d)
        b3_s = singles.tile([128, 1], F32, tag="b3")
        nc.scalar.dma_start(out=b3_s, in_=b3st)
        iota_s = singles.tile([128, A], FP16, tag="iota")
        nc.scalar.dma_start(out=iota_s, in_=iotad)
        actb_s = singles.tile([128, GR * A], I8, tag="actb")
        rewb_s = singles.tile([128, GR], F32, tag="rewb")
        s1b_s = singles.tile([128, GR], F32, tag="s1b")

        # qs: stacked Q^T in SBUF bf16 (stack copies write, dma-transpose reads)
        qs = big.tile([128, NQ, CH], BF16, tag="qs")
        # qbuf: batch-major Q (partition = batch-within-128-block)
        qbuf = big.tile([128, NQ, 4, 128], BF16, tag="qbuf")

        # epilogue tiles
        actf = big.tile([128, GR * A], FP16, tag="actf")
        score = big.tile([128, GR, A], FP16, tag="score")
        rowmax = big.tile([128, GR], FP16, tag="rowmax")
        onehot = big.tile([128, GR, A], BF16, tag="onehot")
        donem = big.tile([128, GR], F32, tag="donem")
        fac = big.tile([128, GR], F32, tag="fac")
        prod = big.tile([128, GR, A], BF16, tag="prod")
        q0sel = big.tile([128, GR], F32, tag="q0sel")
        maxqn = big.tile([128, GR], F32, tag="maxqn")
        t1 = big.tile([128, GR], F32, tag="t1")
        t2 = big.tile([128, GR], F32, tag="t2")
        diff = big.tile([128, GR], F32, tag="diff")
        sq = big.tile([128, GR], F32, tag="sq")
        acc = big.tile([128, 1], F32, tag="acc")

        xL = {}
        h1p_t, h1s_t, h2pa_t, h2pb_t, h2s_t, qt_q = {}, {}, {}, {}, {}, {}

        def do_dma(li):
            x0L = xpool.tile([128, LOADCOLS], FP8, tag="x0")
            x1L = xpool.tile([128, LOADCOLS], FP8, tag="x1")
            nc.sync.dma_start(out=x0L,
                              in_=x0t[:, li * LOADCOLS:(li + 1) * LOADCOLS])
            nc.sync.dma_start(out=x1L,
                              in_=x1t[:, li * LOADCOLS:(li + 1) * LOADCOLS])
            xL[li] = (x0L, x1L)

        def xs_for(t):
            c, pa = t // 2, t % 2
            li = (c * CH) // LOADCOLS
            ci = (c * CH) % LOADCOLS // CH
            return xL[li][pa][:, ci * CH:(ci + 1) * CH]

        def st_mm1(t):
            h1p = ps_h1.tile([128, 2, CH], F32, tag="h1p", name=f"h1p_{t}")
            xs = xs_for(t)
            for m in range(2):
                nc.tensor.matmul(h1p[:, m, :], w1_s[:, m * 128:(m + 1) * 128],
                                 xs, start=True, stop=True)
            h1p_t[t] = h1p

        # relu copy engine alternation
        def relu_engine(idx):
            return nc.scalar if idx % 2 == 0 else nc.vector

        def emit_relu(eng, dst, src, bias_ap):
            if eng is nc.scalar:
                nc.scalar.activation(dst, src, AF.Relu, bias=bias_ap, scale=1.0)
            else:
                nc.vector.tensor_scalar(dst, src, bias_ap, 0.0, OP.add, OP.max)

        def st_relu1(t):
            h1s = h1spool.tile([128, 2, CH], FP8, tag="h1s", name=f"h1s_{t}")
            emit_relu(relu_engine(t),
                      h1s[:, :, :].rearrange("p a b -> p (a b)"),
                      h1p_t.pop(t)[:, :, :].rearrange("p a b -> p (a b)"),
                      b1_s[:, 0:1])
            h1s_t[t] = h1s

        def st_mm2(t):
            h1s = h1s_t.pop(t)
            if USE_DR:
                h2pa = ps_h2.tile([128, CH], F32, tag="h2p", name=f"h2pa_{t}")
                nc.tensor.matmul(h2pa, w2_s[:, :, 0:128], h1s[:, :, :],
                                 start=True, stop=True, perf_mode=DR)
                h2pb = ps_h2.tile([128, CH], F32, tag="h2p", name=f"h2pb_{t}")
                nc.tensor.matmul(h2pb, w2_s[:, :, 128:256], h1s[:, :, :],
                                 start=True, stop=True, perf_mode=DR)
            else:
                h2pa = ps_h2.tile([128, CH], F32, tag="h2p", name=f"h2pa_{t}")
                for k in range(2):
                    nc.tensor.matmul(h2pa, w2_s[:, k, 0:128], h1s[:, k, :],
                                     start=(k == 0), stop=(k == 1))
                h2pb = ps_h2.tile([128, CH], F32, tag="h2p", name=f"h2pb_{t}")
                for k in range(2):
                    nc.tensor.matmul(h2pb, w2_s[:, k, 128:256], h1s[:, k, :],
                                     start=(k == 0), stop=(k == 1))
            h2pa_t[t] = h2pa
            h2pb_t[t] = h2pb

        def st_relu2(t):
            h2s = h2spool.tile([128, 2, CH], FP8, tag="h2s", name=f"h2s_{t}")
            # m0 on engine (t+1)%2, m1 on engine t%2 (anti-collide with relu1)
            emit_relu(relu_engine(t + 1), h2s[:, 0, :], h2pa_t.pop(t),
                      b2_s[:, 0:1])
            emit_relu(relu_engine(t), h2s[:, 1, :], h2pb_t.pop(t),
                      b2_s[:, 1:2])
            h2s_t[t] = h2s

        def st_mm3(c):
            # chunk-pair c: Q0 from h2s[2c] (state0), Qn from h2s[2c+1]
            q = c // 2
            gp = (c % 2) * 2  # group pair base (0 or 2)
            if q not in qt_q:
                qt_q[q] = ps_qt.tile([128, CH], F32, tag="qt", name=f"qt_{q}")
            qt = qt_q[q]
            h2s0 = h2s_t.pop(2 * c)
            h2s1 = h2s_t.pop(2 * c + 1)
            # col-tiled: Q0 and Qn MMs run concurrently on different 32-col
            # groups of the PE array (DoubleRow is illegal off col-group 0)
            for k in range(2):
                for gi, h2sx in ((gp, h2s0), (gp + 1, h2s1)):
                    po = gi * 32
                    nc.tensor.matmul(qt[po:po + A, :], w3_s[:, k, 0:A],
                                     h2sx[:, k, :], start=(k == 0),
                                     stop=(k == 1), tile_position=(0, po))

        def st_stack(q):
            # PSUM f32 -> SBUF bf16 with b3 bias (per stacked partition)
            eng = relu_engine(q)
            if eng is nc.scalar:
                nc.scalar.activation(qs[:, q, :], qt_q[q], AF.Identity,
                                     bias=b3_s[:, 0:1], scale=1.0)
            else:
                nc.vector.tensor_scalar(qs[:, q, :], qt_q[q], b3_s[:, 0:1],
                                        None, OP.add)
            qt_q.pop(q)

        def st_dmaT(q):
            nc.sync.dma_start_transpose(out=qbuf[:, q, :, :], in_=qs[:, q, :])

        def ep_front():
            # argmax/onehot of actions; done mask; discount factor
            a3 = actf[:, :].rearrange("p (g a) -> p g a", a=A)
            iot_b = iota_s[:, None, :].broadcast_to([128, GR, A])
            nc.scalar.activation(actf, actb_s, AF.Copy, scale=32.0)
            nc.vector.tensor_tensor(score, a3, iot_b, OP.subtract)
            nc.vector.tensor_reduce(rowmax, score, AX.X, OP.max)
            nc.vector.tensor_tensor(
                onehot, score, rowmax[:, :, None].broadcast_to([128, GR, A]),
                OP.is_equal)
            nc.vector.tensor_scalar(donem, s1b_s, DONE, None, OP.is_equal)
            nc.vector.tensor_scalar(fac, donem, -DISC, DISC, OP.mult, OP.add)

        def ep_tail(half):
            # half = 0: q 0..3 (blocks 0..31); half = 1: q 4..7
            q0_ = half * 4
            qh = qbuf[:, q0_:q0_ + 4, :, :]          # [128, 4, 4, 128]
            gsl = slice(half * 32, (half + 1) * 32)
            oh3 = onehot[:, :, :]                     # [128, GR, A]
            for pair in range(2):
                # blocks g' = 8q + 2w + pair; qbuf group = 2*pair (+1 Qn)
                q0ap = qh[:, :, :, 64 * pair:64 * pair + A]       # [128,4,4,A]
                qnap = qh[:, :, :, 64 * pair + 32:64 * pair + 32 + A]
                ohap = oh3[:, half * 32 + pair:(half + 1) * 32:2, :] \
                    .rearrange("p (q w) a -> p q w a", q=4)
                prap = prod[:, half * 32 + pair:(half + 1) * 32:2, :] \
                    .rearrange("p (q w) a -> p q w a", q=4)
                nc.vector.tensor_tensor(prap, ohap, q0ap, OP.mult)
                nc.vector.tensor_reduce(
                    q0sel[:, half * 32 + pair:(half + 1) * 32:2]
                    .rearrange("p (q w) -> p q w", q=4), prap, AX.X, OP.add)
                nc.vector.tensor_reduce(
                    maxqn[:, half * 32 + pair:(half + 1) * 32:2]
                    .rearrange("p (q w) -> p q w", q=4), qnap, AX.X, OP.max)
            nc.vector.tensor_tensor(t1[:, gsl], maxqn[:, gsl], fac[:, gsl],
                                    OP.mult)
            nc.vector.tensor_tensor(t2[:, gsl], t1[:, gsl], rewb_s[:, gsl],
                                    OP.add)
            nc.vector.tensor_tensor(diff[:, gsl], q0sel[:, gsl], t2[:, gsl],
                                    OP.subtract)
            nc.vector.tensor_tensor(sq[:, gsl], diff[:, gsl], diff[:, gsl],
                                    OP.mult)

        # ---- main software-pipelined loop ----
        do_dma(0)
        PASS_PER_LOAD = 2 * LOADCOLS // CH   # ticks covered per load pair
        for t in range(T + 10):
            nt = t + 4
            if nt < T and nt % PASS_PER_LOAD == 0:
                do_dma(nt // PASS_PER_LOAD)
            if t == 3:
                nc.sync.dma_start(out=actb_s, in_=actb)
                nc.sync.dma_start(out=rewb_s, in_=rewb)
                nc.sync.dma_start(out=s1b_s, in_=s1b)
            if t == 7:
                ep_front()
            if t < T:
                st_mm1(t)
            if 0 <= t - 2 < T:
                st_mm2(t - 2)
            if 0 <= t - 5 and (t - 5) % 2 == 0 and (t - 5) // 2 < T // 2:
                st_mm3((t - 5) // 2)
            if 0 <= t - 3 < T:
                st_relu2(t - 3)
            if 0 <= t - 1 < T:
                st_relu1(t - 1)
            if t >= 8 and (t - 8) % 4 == 0:
                q = (t - 8) // 4
                if q < NQ:
                    st_stack(q)
            if t >= 9 and (t - 9) % 4 == 0:
                q = (t - 9) // 4
                if q < NQ:
                    st_dmaT(q)
            if t == 23:
                ep_tail(0)
        ep_tail(1)
        nc.vector.tensor_reduce(acc, sq, AX.X, OP.add)
        nc.sync.dma_start(out=outp, in_=acc)

    nc.compile()
    return nc


_CACHE = {}


def _get_program():
    if "nc" not in _CACHE:
        _CACHE["nc"] = _build_program()
    return _CACHE["nc"]


def _block_perm():
    # qbuf block order g' -> source batch block b
    perm = np.empty(GR, np.int64)
    for gp in range(GR):
        q, r = divmod(gp, 8)
        w, pair = divmod(r, 2)
        perm[gp] = (2 * q + pair) * 4 + w
    return perm


def _prep_in_maps(inputs):
    st0 = np.asarray(inputs["states0"], dtype=np.float32)
    st1 = np.asarray(inputs["states1"], dtype=np.float32)
    act = np.asarray(inputs["actions"], dtype=np.int32)
    rew = np.asarray(inputs["rewards"], dtype=np.float32)
    W1 = np.asarray(inputs["W1"], dtype=np.float32)
    W2 = np.asarray(inputs["W2"], dtype=np.float32)
    W3 = np.asarray(inputs["W3"], dtype=np.float32)
    b1 = np.asarray(inputs["b1"], dtype=np.float32)
    b2 = np.asarray(inputs["b2"], dtype=np.float32)
    b3 = np.asarray(inputs["b3"], dtype=np.float32)

    # sanitize DONE sentinel (666 > fp8e4m3 max); done rows' Qn is masked out
    s1col = st1[:, 0].copy()
    st1f = st1.copy()
    st1f[:, 0] = np.where(s1col == DONE, 0.0, s1col)

    w1f = W1.astype(NP_FP8)
    w2km = np.ascontiguousarray(
        W2.reshape(2, 128, H).transpose(1, 0, 2)).astype(NP_FP8).reshape(128, 2 * H)
    w3p = np.zeros((128, 2, 32), np.float32)
    w3p[:, :, :A] = W3.reshape(2, 128, A).transpose(1, 0, 2)
    w3s = w3p.astype(NP_FP8).reshape(128, 2 * 32)
    b1m = np.ascontiguousarray(b1.reshape(2, 128).T)
    b2m = np.ascontiguousarray(b2.reshape(2, 128).T)
    b3stk = np.zeros((128, 1), np.float32)
    for g in range(4):
        b3stk[g * 32:g * 32 + A, 0] = b3
    iota = np.ascontiguousarray(
        np.broadcast_to(np.arange(A, dtype=np.float16), (128, A)))

    perm = _block_perm()
    act8 = act.astype(np.int8)

    in_maps = []
    for c in range(NCORES):
        r0, r1 = c * BC, (c + 1) * BC
        actc = act8[r0:r1].reshape(GR, 128, A)[perm]
        rewc = rew[r0:r1].reshape(GR, 128)[perm]
        s1c = s1col[r0:r1].reshape(GR, 128)[perm]
        in_maps.append({
            "x0t": np.ascontiguousarray(st0[r0:r1].T).astype(NP_FP8),
            "x1t": np.ascontiguousarray(st1f[r0:r1].T).astype(NP_FP8),
            "actb": np.ascontiguousarray(
                actc.transpose(1, 0, 2).reshape(128, GR * A)),
            "rewb": np.ascontiguousarray(rewc.transpose(1, 0)),
            "s1b": np.ascontiguousarray(s1c.transpose(1, 0)),
            "w1": w1f, "w2km": w2km, "w3s": w3s,
            "b1d": b1m, "b2d": b2m, "b3st": b3stk, "iotad": iota,
        })
    return in_maps


def _run(inputs, trace=False):
    nc = _get_program()
    in_maps = _prep_in_maps(inputs)
    res = run_bass_kernel_spmd(nc, in_maps, core_ids=list(range(NCORES)),
                               trace=trace)
    total = 0.0
    for r in res.results:
        total += float(np.asarray(r["outp"], dtype=np.float64).sum())
    return np.array(np.float32(total)), res


def kernel(**inputs) -> np.ndarray:
    val, _ = _run(inputs, trace=False)
    return val
